# revision 27
# baseline (speedup 1.0000x reference)
"""Trainium2 Bass kernel for BinaryGroupConv block (8-core SPMD, batch-sharded).

For x:(32,256,56,56), w1:(256,64,3,3), w2:(256,256,1,1):
    out = bn1(conv2d(sign(x), sign(w1), s2 p1 g4)) + maxpool3x3s2p1(x)
    x1  = out
    out = bn2(conv2d(sign(out), sign(w2), 1x1)) + x1
with training-mode (batch-stat) BatchNorm -> sync-BN all-reduce across cores.

v5 layout (per core, 4 images = 8 units of 128 channels):
  - binary convs in fp8e4 with DoubleRow matmuls: conv1 processes 2 of the 9
    taps per instruction (tap pairs expressed as rearranged views of the
    padded sign image), conv2 contracts both 128-channel halves in one
    instruction. Halves PE time vs bf16 (PE is pinned at 1.2 GHz by the
    power profile, so instruction count is what matters).
  - weights DMA'd on the ACT ring before anything else; all 8 input loads
    enqueued up-front on the SP ring; gamma/beta fetched as two contiguous
    rows and transposed on PE (per-partition gather DMAs are descriptor-
    bound and would stall the input stream).
  - engine balance: ACT does sign + PSUM evicts + binarize2, DVE does
    row-max + bn_stats + fused scalar_tensor_tensor residual work. Pool
    does only memsets (its elementwise path is ~10x slower than DVE and
    holds the shared SBUF port).
  - sync-BN: 2KB AllReduce of (S, SS); col-max / conv2-evict deferred into
    the AR1 / AR2 latency windows.
"""

import contextlib
import sys

import numpy as np

sys.path.insert(0, "/opt/trn_rl_repo")

import concourse.bass as bass
import concourse.tile as tile
from concourse import bacc, mybir
from concourse.bass import ts
from concourse.bass_utils import run_bass_kernel_spmd
from concourse.masks import make_identity
from concourse.tile import add_dep_helper

F32 = mybir.dt.float32
BF16 = mybir.dt.bfloat16
FP16 = mybir.dt.float16
FP8 = mybir.dt.float8e4
AF = mybir.ActivationFunctionType
OP = mybir.AluOpType
DR = mybir.MatmulPerfMode.DoubleRow

EPS = 1e-5
C = 256
H = 56
HO = 28
PIX = HO * HO  # 784
NCHUNK = 392  # matmul/psum N-tile (14 output rows)
RPC = 14  # output rows per chunk
SC1 = 2.0  # conv1: x-sign +/-1, w-sign +/-0.5 -> y_true = 2*y_q
SC2 = 2.0  # conv2: z-sign +/-1, w-sign +/-0.5 -> y_true = 2*y_q

XF_BUFS = 4
XS_BUFS = 2
N_COLS_INLINE = 0


def build_nc(n_loc: int, n_cores: int):
    nc = bacc.Bacc(
        "TRN2",
        target_bir_lowering=False,
        debug=False,
        enable_asserts=False,
        num_devices=n_cores,
    )
    x_d = nc.dram_tensor("x", [n_loc, C, H, H], F32, kind="ExternalInput").ap()
    w1_d = nc.dram_tensor("w1", [C, 64, 3, 3], F32, kind="ExternalInput").ap()
    w2_d = nc.dram_tensor("w2", [C, C, 1, 1], F32, kind="ExternalInput").ap()
    g1_d = nc.dram_tensor("gamma1", [C], F32, kind="ExternalInput").ap()
    b1_d = nc.dram_tensor("beta1", [C], F32, kind="ExternalInput").ap()
    g2_d = nc.dram_tensor("gamma2", [C], F32, kind="ExternalInput").ap()
    b2_d = nc.dram_tensor("beta2", [C], F32, kind="ExternalInput").ap()
    out_d = nc.dram_tensor("out", [n_loc, C, HO, HO], F32, kind="ExternalOutput").ap()

    with tile.TileContext(nc) as tc:
        kernel_body(
            tc, out_d, x_d, w1_d, w2_d, (g1_d, b1_d, g2_d, b2_d), n_loc, n_cores
        )

    nc.compile()
    return nc


def kernel_body(tc, out_d, x_d, w1_d, w2_d, gb_d, n_loc, n_cores):
    nc = tc.nc
    g1_d, b1_d, g2_d, b2_d = gb_d
    n_units = n_loc * 2
    npix_loc = n_loc * PIX
    npix_glob = npix_loc * n_cores
    XSW = 58  # padded xs row length

    ctx = contextlib.ExitStack()
    with ctx:
        singles = ctx.enter_context(tc.tile_pool(name="singles", bufs=1))
        xf_pool = ctx.enter_context(tc.tile_pool(name="xf", bufs=XF_BUFS))
        xs_pool = ctx.enter_context(tc.tile_pool(name="xs", bufs=XS_BUFS))
        rm_pool = ctx.enter_context(tc.tile_pool(name="rmax", bufs=n_units))
        mp_pool = ctx.enter_context(tc.tile_pool(name="mp", bufs=n_units))
        y1_pool = ctx.enter_context(tc.tile_pool(name="y1", bufs=n_units))
        zs_pool = ctx.enter_context(tc.tile_pool(name="zs", bufs=2))
        y2_pool = ctx.enter_context(tc.tile_pool(name="y2q", bufs=n_units))
        tiny = ctx.enter_context(tc.tile_pool(name="tiny", bufs=1))
        dram = ctx.enter_context(tc.tile_pool(name="dram", bufs=1, space="DRAM"))

        # Dummy AllReduce issued first: the TOPSP collectives firmware only
        # becomes ready ~85us after NEFF start and its first AR costs ~20us
        # extra; this one absorbs both concurrently with phase-1 compute.
        warm = tiny.tile([128, 4], F32, tag="warm", name="warm")
        nc.gpsimd.memset(warm, 0.0)
        cc_warm_in = dram.tile([128, 4], F32, tag="cc_warm_in", name="cc_warm_in")
        cc_warm_out = dram.tile([128, 4], F32, tag="cc_warm_out", name="cc_warm_out")
        nc.sync.dma_start(out=cc_warm_in, in_=warm)
        nc.gpsimd.collective_compute(
            "AllReduce",
            OP.add,
            replica_groups=[list(range(n_cores))],
            ins=[cc_warm_in.opt()],
            outs=[cc_warm_out.opt()],
        )


        # ---- weight DMAs on the ACT ring (never behind the input stream) ----
        w1nat = [singles.tile([128, 64, 9], F32, tag=f"w1nat_{t}", name=f"w1nat_{t}") for t in range(2)]
        for t in range(2):
            nc.scalar.dma_start(
                out=w1nat[t],
                in_=w1_d[ts(t, 128)].rearrange("co ci kh kw -> co ci (kh kw)"),
            )
        w2nat = [singles.tile([128, 256], F32, tag=f"w2nat_{m}", name=f"w2nat_{m}") for m in range(2)]
        for mt in range(2):
            nc.scalar.dma_start(out=w2nat[mt], in_=w2_d[ts(mt, 128), :, 0, 0])

        # gamma/beta: two contiguous 128-element rows per tensor (2 descriptors
        # each, vs 128 for a per-partition gather), transposed on PE below.
        vecs_raw = singles.tile([8, 128], F32, tag="vecs_raw", name="vecs_raw")
        for i, d_ap in enumerate((g1_d, b1_d, g2_d, b2_d)):
            src = bass.AP(
                tensor=d_ap.tensor, offset=d_ap.offset, ap=[[128, 2], [1, 128]]
            )
            nc.scalar.dma_start(out=vecs_raw[2 * i : 2 * i + 2, :], in_=src)

        # ---- all input loads enqueued up-front on the SP ring ----
        xf_tiles = []
        for u in range(n_units):
            n, t = u // 2, u % 2
            xf = xf_pool.tile([128, H, H], F32, tag="xf", name=f"xf_{u}")
            nc.sync.dma_start(out=xf, in_=x_d[n, ts(t, 128)])
            xf_tiles.append(xf)

        # ---------------- weight / param prep ----------------
        lhsT1 = [singles.tile([128, 9, 128], FP8, tag=f"lhsT1_{t}", name=f"lhsT1_{t}") for t in range(2)]
        w2dr = [singles.tile([128, 2, 128], FP8, tag=f"w2dr_{m}", name=f"w2dr_{m}") for m in range(2)]
        vecs = singles.tile([128, 8], F32, tag="vecs", name="vecs")
        with tc.tile_pool(name="wprep", bufs=1) as wprep, tc.tile_pool(
            name="tr_psum", bufs=2, space="PSUM"
        ) as tr_psum:
            ident = singles.tile([128, 128], BF16)
            make_identity(nc, ident)
            ident8 = singles.tile([8, 8], F32, tag="ident8", name="ident8")
            make_identity(nc, ident8)
            trv = tr_psum.tile([128, 8], F32, tag="trv", name="trv")
            nc.tensor.transpose(trv, vecs_raw, ident8)
            nc.scalar.copy(out=vecs, in_=trv)

            for t in range(2):
                w1ns = wprep.tile([128, 64, 9], BF16, tag="w1ns", name=f"w1ns_{t}")
                nc.vector.tensor_scalar(
                    out=w1ns, in0=w1nat[t], scalar1=0.0, scalar2=0.5,
                    op0=OP.is_ge, op1=OP.subtract,
                )
                nc.gpsimd.memset(lhsT1[t], 0.0)
                for tap in range(9):
                    trf = tr_psum.tile([128, 128], BF16, tag="trw", name=f"trw_{t}_{tap}")
                    nc.tensor.transpose(trf[0:64], w1ns[:, :, tap], ident)
                    nc.tensor.transpose(trf[64:128], w1ns[:, :, tap], ident)
                    nc.scalar.copy(out=lhsT1[t][0:64, tap, 0:64], in_=trf[0:64, 0:64])
                    nc.vector.tensor_copy(
                        out=lhsT1[t][64:128, tap, 64:128], in_=trf[64:128, 64:128]
                    )
            for mt in range(2):
                w2s = wprep.tile([128, 256], BF16, tag="w2s", name=f"w2s_{mt}")
                nc.vector.tensor_scalar(
                    out=w2s, in0=w2nat[mt], scalar1=0.0, scalar2=0.5,
                    op0=OP.is_ge, op1=OP.subtract,
                )
                for kt in range(2):
                    tr = tr_psum.tile([128, 128], BF16, tag="trw", name=f"tr_{mt}_{kt}")
                    nc.tensor.transpose(tr, w2s[:, ts(kt, 128)], ident)
                    nc.scalar.copy(out=w2dr[mt][:, kt, :], in_=tr)

        # per-partition affine params: vecs columns [g1_0,g1_1,b1_0,b1_1,...]
        g1_t = [vecs[:, 0 + t : 1 + t] for t in range(2)]
        b1_t = [vecs[:, 2 + t : 3 + t] for t in range(2)]
        g2_t = [vecs[:, 4 + t : 5 + t] for t in range(2)]
        b2_t = [vecs[:, 6 + t : 7 + t] for t in range(2)]

        eps_t = singles.tile([128, 1], F32)
        nc.vector.memset(eps_t, EPS)

        bnst1 = [
            singles.tile([128, n_units, 6], F32, tag=f"bnst1_{t}", name=f"bnst1_{t}") for t in range(2)
        ]
        bnst2 = [
            singles.tile([128, n_units, 6], F32, tag=f"bnst2_{t}", name=f"bnst2_{t}") for t in range(2)
        ]

        # ------- phase 1 stage functions -------
        xs_tiles = {}
        rm_tiles = {}
        mp_tiles = {}
        y1_tiles = {}
        ps_tiles = {}

        def st_sign(u):
            xf = xf_tiles[u]
            xs = xs_pool.tile([128, H + 1, XSW], FP8)
            if u < XS_BUFS:
                nc.gpsimd.memset(xs[:, 0, :], 0.0)
                nc.gpsimd.memset(xs[:, 1:, 1], 0.0)
                nc.gpsimd.memset(xs[:, 1:, 0], 0.0)
            nc.scalar.sign(out=xs[:, 1:, 2:58], in_=xf)
            xs_tiles[u] = xs

        def st_rows(u):
            xf = xf_tiles[u]
            rmax = rm_pool.tile([128, HO, H], F32)
            nc.vector.tensor_tensor(
                out=rmax, in0=xf[:, 0:H:2], in1=xf[:, 1:H:2], op=OP.max
            )
            nc.vector.tensor_tensor(
                out=rmax[:, 1:], in0=rmax[:, 1:], in1=xf[:, 1 : H - 2 : 2], op=OP.max
            )
            rm_tiles[u] = rmax

        def st_conv(u, psum1):
            # 9 taps as 4 fp8 DoubleRow matmuls + 1 plain, per 392-pixel chunk.
            t = u % 2
            xs = xs_tiles[u]
            ps = [
                psum1.tile([128, RPC, HO], F32, tag=f"ps1_{c}", name=f"ps1_{u}_{c}")
                for c in range(2)
            ]
            # (lhsT tap slice, is_first) per DR pair; c-inner keeps the
            # stationary operand resident for both chunks.
            for pi, (ta, step) in enumerate([(0, 1), (3, 1), (6, 1), (2, 3)]):
                lh = lhsT1[t][:, ta : ta + step + 1 : step, :]
                for c in range(2):
                    r0 = 28 * c
                    if step == 1:  # taps (kh,0)+(kh,1): col-pair factorization
                        kh = ta // 3
                        base = xs[:, r0 + kh : r0 + kh + 27 : 2, 1:57]
                        rhs = base.rearrange("p r (k two) -> p two r k", two=2)
                    else:  # taps (0,2)+(1,2): row-pair factorization
                        base = xs[:, r0 : r0 + 28, 3:58:2]
                        rhs = base.rearrange("p (r two) k -> p two r k", two=2)
                    nc.tensor.matmul(
                        ps[c], lh, rhs, start=(pi == 0), stop=False, perf_mode=DR
                    )
            for c in range(2):  # tap (2,2) plain fp8 matmul
                r0 = 28 * c
                rhs = xs[:, r0 + 2 : r0 + 29 : 2, 3:58:2]
                nc.tensor.matmul(
                    ps[c], lhsT1[t][:, 8, :], rhs, start=False, stop=True
                )
            ps_tiles[u] = ps

        def st_evict(u):
            ps = ps_tiles.pop(u)
            y1 = y1_pool.tile([128, PIX], F32)
            for c in range(2):
                nc.scalar.copy(
                    out=y1[:, ts(c, NCHUNK)].rearrange("p (a b) -> p a b", a=RPC),
                    in_=ps[c],
                )
            y1_tiles[u] = y1

        def st_stats1(u):
            n, t = u // 2, u % 2
            y1 = y1_tiles[u]
            for c in range(2):
                nc.vector.bn_stats(
                    out=bnst1[t][:, 2 * n + c, :], in_=y1[:, ts(c, NCHUNK)]
                )

        def st_cols(u):
            rmax = rm_tiles[u]
            mp = mp_pool.tile([128, HO, HO], F32, tag="mp", name=f"mp_{u}")
            nc.vector.tensor_tensor(
                out=mp, in0=rmax[:, :, 0:H:2], in1=rmax[:, :, 1:H:2], op=OP.max
            )
            nc.vector.tensor_tensor(
                out=mp[:, :, 1:], in0=mp[:, :, 1:],
                in1=rmax[:, :, 1 : H - 2 : 2], op=OP.max,
            )
            mp_tiles[u] = mp

        # ------- phase 1: software-pipelined emission -------
        with tc.tile_pool(name="psum1", bufs=4, space="PSUM") as psum1:
            for u in range(n_units):
                st_sign(u)
                st_rows(u)
                st_conv(u, psum1)
                if u >= 1:
                    st_evict(u - 1)
                    if u - 1 < N_COLS_INLINE:
                        st_cols(u - 1)
            st_evict(n_units - 1)
            for u in range(n_units):
                st_stats1(u)

            # ---- local aggregate -> (S, SS) -> AllReduce ----
            # The TOPSP collectives firmware only becomes ready ~85us after
            # NEFF start; no warmup AR is issued since phase 1 ends earlier
            # anyway and a warmup would only occupy the CC core when the real
            # AR1 wants it.
            def ar_launch(allin, tag):
                cc_in = dram.tile([128, 4], F32, tag=f"ccin_{tag}", name=f"ccin_{tag}")
                cc_out = dram.tile([128, 4], F32, tag=f"ccout_{tag}", name=f"ccout_{tag}")
                nc.sync.dma_start(out=cc_in, in_=allin)
                nc.gpsimd.collective_compute(
                    "AllReduce",
                    OP.add,
                    replica_groups=[list(range(n_cores))],
                    ins=[cc_in.opt()],
                    outs=[cc_out.opt()],
                )
                gst = tiny.tile([128, 4], F32, tag=f"gst_{tag}", name=f"gst_{tag}")
                nc.sync.dma_start(out=gst, in_=cc_out)
                return gst

            def stats_allreduce(bnst, tag):
                allin = tiny.tile([128, 4], F32, tag=f"allin_{tag}", name=f"allin_{tag}")
                for t in range(2):
                    mv = tiny.tile([128, 2], F32, tag=f"mv_{tag}_{t}", name=f"mv_{tag}_{t}")
                    nc.vector.bn_aggr(out=mv, in_=bnst[t])
                    m2 = tiny.tile([128, 1], F32, tag=f"m2_{tag}_{t}", name=f"m2_{tag}_{t}")
                    nc.vector.tensor_tensor(
                        out=m2, in0=mv[:, 0:1], in1=mv[:, 0:1], op=OP.mult
                    )
                    vp = tiny.tile([128, 1], F32, tag=f"vp_{tag}_{t}", name=f"vp_{tag}_{t}")
                    nc.vector.tensor_tensor(out=vp, in0=mv[:, 1:2], in1=m2, op=OP.add)
                    nc.vector.tensor_scalar_mul(
                        out=allin[:, 2 * t : 2 * t + 1], in0=mv[:, 0:1],
                        scalar1=float(npix_loc),
                    )
                    nc.vector.tensor_scalar_mul(
                        out=allin[:, 2 * t + 1 : 2 * t + 2], in0=vp,
                        scalar1=float(npix_loc),
                    )
                return ar_launch(allin, tag)

            gst1 = stats_allreduce(bnst1, "s1")

            # deferred col-max of the last units runs during the AR1 flight
            for u in range(N_COLS_INLINE, n_units):
                st_cols(u)

            def bn_coeffs(gst, gam2, bet2, tag, SC):
                """Global (S,SS) -> (a_eff, b_eff) for both part-tiles at once:
                out = y_q*a_eff + b_eff. gst cols = [S0, SS0, S1, SS1]."""
                mq = tiny.tile([128, 2], F32, tag=f"mq_{tag}", name=f"mq_{tag}")
                nc.vector.tensor_scalar_mul(
                    out=mq, in0=gst[:, 0:3:2], scalar1=1.0 / npix_glob
                )
                sq = tiny.tile([128, 2], F32, tag=f"sq_{tag}", name=f"sq_{tag}")
                nc.vector.tensor_scalar_mul(
                    out=sq, in0=gst[:, 1:4:2], scalar1=1.0 / npix_glob
                )
                m2 = tiny.tile([128, 2], F32, tag=f"cm2_{tag}", name=f"cm2_{tag}")
                nc.vector.tensor_tensor(out=m2, in0=mq, in1=mq, op=OP.mult)
                vq = tiny.tile([128, 2], F32, tag=f"varq_{tag}", name=f"varq_{tag}")
                nc.vector.tensor_tensor(out=vq, in0=sq, in1=m2, op=OP.subtract)
                vt = tiny.tile([128, 2], F32, tag=f"vart_{tag}", name=f"vart_{tag}")
                nc.vector.tensor_scalar_mul(out=vt, in0=vq, scalar1=SC * SC)
                sd = tiny.tile([128, 2], F32, tag=f"sd_{tag}", name=f"sd_{tag}")
                nc.scalar.activation(out=sd, in_=vt, func=AF.Sqrt, bias=eps_t)
                r = tiny.tile([128, 2], F32, tag=f"r_{tag}", name=f"r_{tag}")
                nc.vector.reciprocal(out=r, in_=sd)
                rg = tiny.tile([128, 2], F32, tag=f"rg_{tag}", name=f"rg_{tag}")
                nc.vector.tensor_tensor(out=rg, in0=r, in1=gam2, op=OP.mult)
                a_eff = tiny.tile([128, 2], F32, tag=f"aeff_{tag}", name=f"aeff_{tag}")
                nc.vector.tensor_scalar_mul(out=a_eff, in0=rg, scalar1=SC)
                mrg = tiny.tile([128, 2], F32, tag=f"mrg_{tag}", name=f"mrg_{tag}")
                nc.vector.tensor_tensor(out=mrg, in0=mq, in1=rg, op=OP.mult)
                b_eff = tiny.tile([128, 2], F32, tag=f"beff_{tag}", name=f"beff_{tag}")
                nc.vector.scalar_tensor_tensor(
                    out=b_eff, in0=mrg, scalar=-SC, in1=bet2,
                    op0=OP.mult, op1=OP.add,
                )
                return [(a_eff[:, t : t + 1], b_eff[:, t : t + 1]) for t in range(2)]

            c1 = bn_coeffs(gst1, vecs[:, 0:2], vecs[:, 2:4], "s1", SC1)


        # ------- phase 2: q = a1*y1 + mp, zs = sign(q + b1), conv2, stats -------
        # x1 = q + b1_eff; binarize folds b1_eff into the ACT sign bias, the
        # residual picks it up in phase 3 as a (b1_eff + b2_eff) bias.
        zs_imgs = {}
        y2_tiles = {}

        def st_q(u):
            t = u % 2
            y1 = y1_tiles[u]
            a_eff, _ = c1[t]
            nc.vector.scalar_tensor_tensor(
                out=y1, in0=y1, scalar=a_eff,
                in1=mp_tiles[u].rearrange("p a b -> p (a b)"),
                op0=OP.mult, op1=OP.add,
            )
            # y1 now holds q = a1*y1 + mp

        def st_zs(u):
            n, t = u // 2, u % 2
            if t == 0:
                zs_imgs[n] = zs_pool.tile([128, 2, PIX], FP8, tag="zs", name=f"zs_{n}")
            _, b_eff = c1[t]
            nc.scalar.sign(out=zs_imgs[n][:, t, :], in_=y1_tiles[u], bias=b_eff)

        def st_conv2(n, mt, psum2):
            ps = [
                psum2.tile([128, NCHUNK], F32, tag=f"ps2_{c}", name=f"ps2_{n}_{mt}_{c}")
                for c in range(2)
            ]
            for c in range(2):
                nc.tensor.matmul(
                    ps[c], w2dr[mt], zs_imgs[n][:, :, ts(c, NCHUNK)],
                    start=True, stop=True, perf_mode=DR,
                )
            ps_tiles[(n, mt)] = ps

        def st_evict2(n, mt):
            ps = ps_tiles.pop((n, mt))
            y2 = y2_pool.tile([128, PIX], FP16, tag="y2q", name=f"y2q_{n}_{mt}")
            for c in range(2):
                nc.scalar.copy(out=y2[:, ts(c, NCHUNK)], in_=ps[c])
            y2_tiles[(n, mt)] = y2

        def st_stats2(n, mt):
            y2 = y2_tiles[(n, mt)]
            for c in range(2):
                nc.vector.bn_stats(
                    out=bnst2[mt][:, 2 * n + c, :], in_=y2[:, ts(c, NCHUNK)]
                )

        with tc.tile_pool(name="psum2", bufs=4, space="PSUM") as psum2:
            for n in range(n_loc):
                for t in range(2):
                    st_q(2 * n + t)
                    st_zs(2 * n + t)
                for mt in range(2):
                    st_conv2(n, mt, psum2)
                if n >= 1:
                    for mt in range(2):
                        st_evict2(n - 1, mt)
                        st_stats2(n - 1, mt)
            for mt in range(2):
                st_evict2(n_loc - 1, mt)
                st_stats2(n_loc - 1, mt)

            gst2 = stats_allreduce(bnst2, "s2")

            c2 = bn_coeffs(gst2, vecs[:, 4:6], vecs[:, 6:8], "s2", SC2)

            # combined bias for phase 3: b1_eff + b2_eff per output tile
            b12 = []
            for mt in range(2):
                b = tiny.tile([128, 1], F32, tag=f"b12_{mt}", name=f"b12_{mt}")
                nc.vector.tensor_tensor(
                    out=b, in0=c1[mt][1], in1=c2[mt][1], op=OP.add
                )
                b12.append(b)

            # ------- phase 3: out = a2*y2q + q + (b1+b2), store -------
            # dead mp tiles are reused as fp32 staging buffers for the store
            for n in range(n_loc):
                for mt in range(2):
                    u = 2 * n + mt
                    a_eff, _ = c2[mt]
                    ob = mp_tiles[u].rearrange("p a b -> p (a b)")
                    nc.vector.scalar_tensor_tensor(
                        out=ob, in0=y2_tiles[(n, mt)], scalar=a_eff,
                        in1=y1_tiles[u], op0=OP.mult, op1=OP.add,
                    )
                    nc.scalar.activation(
                        out=ob, in_=ob, func=AF.Identity, bias=b12[mt]
                    )
                    eng = nc.sync if mt == 0 else nc.scalar
                    eng.dma_start(
                        out=out_d[n, ts(mt, 128)],
                        in_=ob.rearrange("p (h w) -> p h w", h=HO),
                    )


_NC_CACHE = {}


def get_nc(n_loc=4, n_cores=8):
    key = (n_loc, n_cores)
    if key not in _NC_CACHE:
        _NC_CACHE[key] = build_nc(n_loc, n_cores)
    return _NC_CACHE[key]


def kernel(**inputs):
    n_cores = 8
    x = np.asarray(inputs["x"], dtype=np.float32)
    n_loc = x.shape[0] // n_cores
    nc = get_nc(n_loc, n_cores)
    shared = {
        k: np.asarray(inputs[k], dtype=np.float32)
        for k in ("w1", "w2", "gamma1", "beta1", "gamma2", "beta2")
    }
    in_maps = [{"x": x[i * n_loc : (i + 1) * n_loc], **shared} for i in range(n_cores)]
    res = run_bass_kernel_spmd(nc, in_maps, core_ids=list(range(n_cores)))
    return np.concatenate([res.results[i]["out"] for i in range(n_cores)], axis=0)
